# revision 40
# baseline (speedup 1.0000x reference)
"""Trainium2 Bass kernel for the DiCNN (WaveNet-like) module.

Sharding: pure data parallelism — 4 batch items per core on 8 cores.
On-chip layout: channels on partitions, time on the free dim; the four
batch items are stacked as 4x32-partition bands (block-diag weights).

Structure: a software-pipelined tile-major loop over eight 512-wide
time tiles. The output stage of tile t-1 is woven through tile t's
body at every dependency stall point, so the PE always has long-ready
matmuls queued where it would otherwise idle waiting on the
scalar/vector activation chain. A dependency-free matmul warm-up burst
overlaps the input DMA-transposes to release the PE HAM clock gate
early.

PE-array tiling (the big cycle saves):
- The causal conv runs as 4 COL-TILED concurrent matmuls (one per
  batch, tile_position=(0, 32b)): each batch's input is staged as a
  128-partition stack [x_t | x_{t-1}], so one 512-cycle span computes
  what used to take four.
- The final 32->448 conv runs as ROW-TILED concurrent pairs: band q's
  s1 lives at partition strip 32q of one [128, T] tile, so bands can
  share the array in 32-row strips (K=32 each, tile_position=(32q, 0)),
  two at a time into one 2-bank PSUM tile.

The final conv is "flipped": stationary = s1 chunk [32, 128], moving =
w_sk2^T replicated on each 32-row strip [32, 448]; PSUM holds [t, co]
— the output layout. The stationary columns take times at stride 4
(t = tc0 + 4p + j for sub-chunk j), which makes a band's full 512-step
tile [128p, 4j, 448c] exactly match the row-major HBM layout of
y[q, tc0:tc0+512, :] — so each band-tile leaves in ONE store DMA (the
~600ns sync-engine cost per dma_start is flat in transfer size, so
store count is what matters). b_sk2 is added on the host (the bias row
would push K to 33, breaking the 32-row strip packing).

y is stored bf16 (halves store-DMA traffic) and upcast to fp32 on the
host. All matmul operands are bf16 (fp32 PSUM accumulation). x is
pre-transposed to channel-major bf16 on the host so input loads are
plain chunked DMAs (the HWDGE transpose path is ~4x slower and was
crowding the sync ring).
"""

import numpy as np
import ml_dtypes

import concourse.bacc as bacc
import concourse.tile as tile
from concourse import mybir
from concourse.bass_utils import run_bass_kernel_spmd

BF16 = mybir.dt.bfloat16
FP32 = mybir.dt.float32

B, T, C_IN, HID, C_OUT, K = 32, 4096, 64, 32, 448, 2
N_CORES = 8
BPC = B // N_CORES          # batches per core = 4
TT = 512                    # time-tile size
NT = T // TT                # 8 tiles
XROWS = 4112                # 4097 rounded up to a multiple of 16 (xbar rows)
DELTA = 1                   # output-stage pipeline delay in tiles
N_WARMUP = 0                # dependency-free warm-up matmuls

AF = mybir.ActivationFunctionType
ALU = mybir.AluOpType

_cached_nc = None


def _f(x):
    return np.asarray(x, dtype=np.float32)


def _bf(x):
    return np.asarray(x, dtype=np.float32).astype(ml_dtypes.bfloat16)


def _tile4(v):
    return np.tile(_f(v).reshape(-1), 4).reshape(128, 1)


def prepare_weights(w_causal, b_causal, wd0, bd0, ws0, bs0, wo0, bo0,
                    wd1, bd1, ws1, bs1, wo1, bo1, w_sk1, b_sk1, w_sk2, b_sk2):
    """Host-side weight layout transforms (identical for every core)."""
    del wo1, bo1, b_sk2  # wo1/bo1 dead; b_sk2 added host-side

    def diag4(w32):
        s = np.zeros((128, 128), np.float32)
        for i in range(4):
            s[32 * i:32 * i + 32, 32 * i:32 * i + 32] = w32
        return s

    # per-batch causal stationary [128, 32]: rows 0:64 act on x_t
    # (tap k=1), rows 64:128 on x_{t-1} (tap k=0)
    wc = np.zeros((128, 32), np.float32)
    wc[0:64, :] = _f(w_causal)[:, :, 1].T
    wc[64:128, :] = _f(w_causal)[:, :, 0].T

    wd = np.zeros((128, 4, 128), np.float32)
    for blk, w in enumerate((wd0, wd1)):
        for k in range(2):
            wd[:, 2 * blk + k, :] = diag4(_f(w)[:, :, k].T)

    wsr = np.zeros((128, 2, 128), np.float32)
    wsr[:, 0, :] = diag4(_f(ws0)[:, :, 0].T)
    wsr[:, 1, :] = diag4(_f(wo0)[:, :, 0].T)
    ws1d = diag4(_f(ws1)[:, :, 0].T)

    # wsk1[:, qq, :]: M=64 outputs -> s1 bands 2qq (cols 0:32) and
    # 2qq+1 (cols 32:64)
    wsk1 = np.zeros((128, 2, 64), np.float32)
    w1T = _f(w_sk1)[:, :, 0].T
    for qq in range(2):
        wsk1[64 * qq:64 * qq + 32, qq, 0:32] = w1T
        wsk1[64 * qq + 32:64 * qq + 64, qq, 32:64] = w1T

    # w2 replicated on every 32-row strip (row-tiled moving operand)
    w2 = np.zeros((128, 448), np.float32)
    for off in range(0, 128, 32):
        w2[off:off + 32, :] = _f(w_sk2)[:, :, 0].T

    bvecs = np.zeros((128, 6), np.float32)
    bvecs[:, 0] = _tile4(b_causal)[:, 0]
    bvecs[:, 1] = _tile4(bd0)[:, 0]
    bvecs[:, 2] = _tile4(bd1)[:, 0]
    bvecs[:, 3] = _tile4(bo0)[:, 0]
    bvecs[:, 4] = _tile4(_f(bs0) + _f(bs1))[:, 0]
    bvecs[:, 5] = _tile4(b_sk1)[:, 0]

    return dict(
        wc=_bf(wc), wd=_bf(wd), wsr=_bf(wsr), ws1d=_bf(ws1d),
        wsk1=_bf(wsk1), w2=_bf(w2), bvecs=np.ascontiguousarray(bvecs),
    )


def prepare_x(x, core):
    """Per-core channel-major input staging array [BPC, 128, XROWS] bf16.

    Column 1+t of batch b holds [x[b, t, :] | x[b, t-1, :]] — the causal
    two-tap stack — so the causal conv is a single K=128 matmul per
    batch, and the on-chip layout is loaded with plain (non-transposing)
    DMAs. Column 0 (and the t-1 half of column 1) are the causal pad.
    """
    xT = np.zeros((BPC, 128, XROWS), ml_dtypes.bfloat16)
    xb = _bf(x)
    for b in range(BPC):
        xT[b, 0:64, 1:1 + T] = xb[BPC * core + b].T
        xT[b, 64:128, 2:1 + T] = xb[BPC * core + b][:-1].T
    return xT


def build_nc():
    nc = bacc.Bacc("TRN2", target_bir_lowering=False, debug=False,
                   num_devices=N_CORES)

    xT_d = nc.dram_tensor("xT", [BPC, 128, XROWS], BF16,
                          kind="ExternalInput")
    wc_d = nc.dram_tensor("wc", [128, 32], BF16, kind="ExternalInput")
    wd_d = nc.dram_tensor("wd", [128, 4, 128], BF16, kind="ExternalInput")
    wsr_d = nc.dram_tensor("wsr", [128, 2, 128], BF16, kind="ExternalInput")
    ws1_d = nc.dram_tensor("ws1d", [128, 128], BF16, kind="ExternalInput")
    wsk1_d = nc.dram_tensor("wsk1", [128, 2, 64], BF16, kind="ExternalInput")
    w2_d = nc.dram_tensor("w2", [128, 448], BF16, kind="ExternalInput")
    bv_d = nc.dram_tensor("bvecs", [128, 6], FP32, kind="ExternalInput")
    # y[q, jt, p, j, c] is element (b=q, t=512*jt + 4*p + j, c) of the
    # [BPC, T, C_OUT] output — same bytes, viewed 5-D so a whole band-tile
    # leaves in one DMA.
    y_d = nc.dram_tensor("y", [BPC, NT, 128, 4, C_OUT], BF16,
                         kind="ExternalOutput")

    with tile.TileContext(nc) as tc:
        with (
            tc.tile_pool(name="const", bufs=1) as const,
            tc.tile_pool(name="persist", bufs=1) as persist,
            tc.tile_pool(name="act", bufs=3) as actp,
            tc.tile_pool(name="gtile", bufs=2) as gtp,
            tc.tile_pool(name="outbuf", bufs=4) as outbuf,
            tc.tile_pool(name="pg", bufs=4, space="PSUM") as pgp,
            tc.tile_pool(name="pout", bufs=2, space="PSUM") as poutp,
        ):
            # ---- constants (wd first — the warm-up burst needs it) ----
            wd_s = const.tile([128, 4, 128], BF16)
            nc.sync.dma_start(wd_s[:], wd_d.ap())
            wc_s = const.tile([128, 32], BF16)
            nc.sync.dma_start(wc_s[:], wc_d.ap())
            x_s = [persist.tile([128, XROWS], BF16, tag=f"x{b}",
                                name=f"x_s{b}") for b in range(BPC)]
            for b in range(BPC):
                nc.sync.dma_start(x_s[b][:, 0:2 * TT], xT_d[b, :, 0:2 * TT])
            wsr_s = const.tile([128, 2, 128], BF16)
            nc.sync.dma_start(wsr_s[:], wsr_d.ap())
            ws1_s = const.tile([128, 128], BF16)
            nc.sync.dma_start(ws1_s[:], ws1_d.ap())
            wsk1_s = const.tile([128, 2, 64], BF16)
            nc.sync.dma_start(wsk1_s[:], wsk1_d.ap())
            w2_s = const.tile([128, 448], BF16)
            nc.sync.dma_start(w2_s[:], w2_d.ap())
            bv_s = const.tile([128, 6], FP32)
            nc.sync.dma_start(bv_s[:], bv_d.ap())

            bcausal = bv_s[:, 0:1]
            bd_v = (bv_s[:, 1:2], bv_s[:, 2:3])
            bo0_v = bv_s[:, 3:4]
            bskip_v = bv_s[:, 4:5]
            bsk1_v = bv_s[:, 5:6]

            # ---- persistent activations ----
            # x loads are plain copies (host pre-transposed), chunked 1024
            # columns at a time; the first chunks were issued right after
            # wc/wd above so slot 0 starts as early as possible.
            for c0 in list(range(2 * TT, T, 2 * TT)) + [T]:
                cw = 2 * TT if c0 < T else 16
                for b in range(BPC):
                    nc.sync.dma_start(x_s[b][:, c0:c0 + cw],
                                      xT_d[b, :, c0:c0 + cw])
            z0_s = persist.tile([128, 4100], BF16, tag="z0")
            nc.vector.memset(z0_s[:, 0:1], 0.0)
            z1_s = persist.tile([128, 4100], BF16, tag="z1")
            nc.vector.memset(z1_s[:, 0:2], 0.0)
            # s1: band q on partition strip 32q:32q+32
            s1_s = persist.tile([128, T], BF16, tag="s1")

            # ---- PE warm-up burst (overlaps the x-transpose DMAs) ----
            wu_t = persist.tile([128, TT], BF16, tag="wu")
            nc.vector.memset(wu_t[:], 0.0)
            # preload the tanh/sigmoid activation tables so the ~1.3us
            # ACT_TABLE_LOADs happen during the input-DMA wait, not on
            # the first tile's critical chain
            pre_t = actp.tile([128, 1], BF16, tag="a", name="tbl_warm")
            nc.scalar.activation(pre_t[:], wu_t[:, 0:1], AF.Tanh)
            nc.scalar.activation(pre_t[:], wu_t[:, 0:1], AF.Sigmoid)
            hb_cnt = [0]

            def heartbeat(n):
                """Dependency-free PE filler matmuls: keep the HAM activity
                window busy across short dependency stalls so the 2.4 GHz
                clock state is never lost."""
                for _ in range(n):
                    pwu = poutp.tile([128, 2, TT], FP32, tag="po",
                                     name=f"pwu_{hb_cnt[0]}")
                    hb_cnt[0] += 1
                    nc.tensor.matmul(pwu[:, 0, :], wd_s[:, 0, :], wu_t[:],
                                     start=True, stop=True)

            heartbeat(N_WARMUP)

            # ---- woven output stage ------------------------------------
            # Source tile jt: 8 groups g = 4*bp + j (band-pair bp, chunk
            # j). Group g: 2 row-tiled concurrent MMs for bands 2bp,
            # 2bp+1, sub-chunk j: out rows t = tc0 + 4p + j.
            def emit_out_mms(jt, bp, j):
                tc0 = TT * jt
                po = poutp.tile([128, 2, TT], FP32, tag="po",
                                name=f"po_{jt}_{bp}_{j}")
                for i in range(2):
                    off = 32 * (2 * bp + i)
                    nc.tensor.matmul(
                        po[:, i, 0:C_OUT],
                        s1_s[off:off + 32, tc0 + j:tc0 + j + 509:4],
                        w2_s[off:off + 32, :], start=True, stop=True,
                        tile_position=(off, 0))
                return po

            def emit_out_drain(jt, bp, j, po, o_t, eng):
                if eng == 0:
                    nc.scalar.copy(o_t[:, j, :, :], po[:, :, 0:C_OUT])
                elif eng == 1:
                    nc.vector.tensor_copy(o_t[:, j, :, :], po[:, :, 0:C_OUT])
                else:
                    half = C_OUT // 2
                    nc.scalar.copy(o_t[:, j, :, 0:half], po[:, :, 0:half])
                    nc.vector.tensor_copy(o_t[:, j, :, half:C_OUT],
                                          po[:, :, half:C_OUT])
                if j == 3:
                    for i in range(2):
                        nc.sync.dma_start(y_d[2 * bp + i, jt],
                                          o_t[:, :, i, :])

            class Weaver:
                """Persistent output-stage scheduler. Tiles push their 8
                groups when their s1 write has been emitted; step() at a
                PE stall point enqueues the next group's matmuls, first
                emitting the cast of the group TWO steps back (its MMs
                are ~2.5us old by then, so the cast never stalls at the
                head of its engine queue). Casts alternate engines."""

                def __init__(self):
                    self.pend = []
                    self.next_mm = 0
                    self.next_drain = 0
                    self.inflight = {}
                    self.obufs = {}

                def push_tile(self, jt):
                    for j in range(4):
                        for bp in range(2):
                            self.pend.append((jt, bp, j))

                def _drain_until(self, stop, eng=None):
                    while self.next_drain < stop:
                        g = self.next_drain
                        jt, bp, j = self.pend[g]
                        emit_out_drain(jt, bp, j, self.inflight.pop(g),
                                       self.obufs[(jt, bp)],
                                       g % 2 if eng is None else eng)
                        if j == 3:
                            del self.obufs[(jt, bp)]
                        self.next_drain += 1

                def step(self, n=1):
                    for _ in range(n):
                        if self.next_mm >= len(self.pend):
                            return
                        # lag-2 drain; also satisfies the 2-buf pout pool
                        self._drain_until(max(self.next_drain,
                                              self.next_mm - 1))
                        g = self.next_mm
                        jt, bp, j = self.pend[g]
                        if j == 0:
                            self.obufs[(jt, bp)] = outbuf.tile(
                                [128, 4, 2, C_OUT], BF16, tag="o",
                                name=f"o_{jt}_{bp}")
                        self.inflight[g] = emit_out_mms(jt, bp, j)
                        self.next_mm += 1

                def idle(self):
                    return self.next_mm >= len(self.pend)

                def flush(self):
                    while self.next_mm < len(self.pend):
                        self.step(1)
                        self._drain_until(self.next_mm - 1, eng=2)
                    self._drain_until(self.next_mm, eng=2)

            # Two-stage software pipeline: slot s runs block-0 of tile s
            # (H1) interleaved with block-1+head of tile s-1 (H2) and the
            # woven output stage of tile s-2. Two dependency chains in
            # flight halve the wall-clock cost of the serial
            # z0->g0->z1->g1->s0->s1 chain.
            state = {}

            def emit_slot(s, wv):
                t0 = TT * s            # H1 tile offset
                u0 = TT * (s - 1)      # H2 tile offset
                h1 = s < NT
                h2 = 1 <= s <= NT
                st = state.get(s - 1)

                if h1:
                    # causal conv: 4 col-tiled concurrent MMs -> pz
                    pz = pgp.tile([128, TT], FP32, tag="ps", name=f"pz_{s}")
                    for b in range(BPC):
                        nc.tensor.matmul(pz[32 * b:32 * b + 32, :], wc_s[:],
                                         x_s[b][:, 1 + t0:1 + t0 + TT],
                                         start=True, stop=True,
                                         tile_position=(0, 32 * b))
                wv.step(1)                                    # W0
                if h2:
                    pg1 = pgp.tile([128, TT], FP32, tag="ps", name=f"pg1_{s}")
                    nc.tensor.matmul(pg1[:], wd_s[:, 2, :],
                                     z1_s[:, u0:u0 + TT],
                                     start=True, stop=False)
                    nc.tensor.matmul(pg1[:], wd_s[:, 3, :],
                                     z1_s[:, 2 + u0:2 + u0 + TT],
                                     start=False, stop=True)
                if h1:
                    nc.vector.tensor_scalar_add(z0_s[:, 1 + t0:1 + t0 + TT],
                                                pz[:], bcausal)
                if wv.idle():
                    heartbeat(4)
                wv.step(1)                                    # W1
                if h2:
                    a1 = actp.tile([128, TT], BF16, tag="a", name=f"a1_{s}")
                    nc.scalar.activation(a1[:], pg1[:], AF.Tanh, bias=bd_v[1])
                    b1 = actp.tile([128, TT], BF16, tag="b", name=f"b1_{s}")
                    nc.scalar.activation(b1[:], pg1[:], AF.Sigmoid,
                                         bias=bd_v[1])
                if h1:
                    pg0 = pgp.tile([128, TT], FP32, tag="ps", name=f"pg0_{s}")
                    nc.tensor.matmul(pg0[:], wd_s[:, 0, :],
                                     z0_s[:, t0:t0 + TT],
                                     start=True, stop=False)
                    nc.tensor.matmul(pg0[:], wd_s[:, 1, :],
                                     z0_s[:, 1 + t0:1 + t0 + TT],
                                     start=False, stop=True)
                if h2:
                    g1 = gtp.tile([128, TT], BF16, tag="g1", name=f"g1_{s}")
                    nc.vector.tensor_mul(g1[:], a1[:], b1[:])
                wv.step(1)                                    # W2
                if h2:
                    # head: s0 = relu(skip0 + ws1@g1 + bias); ws1 matmul
                    # accumulates onto psA's bank
                    nc.tensor.matmul(st["psA"][:], ws1_s[:], g1[:],
                                     start=False, stop=True)
                if h1:
                    a0 = actp.tile([128, TT], BF16, tag="a", name=f"a0_{s}")
                    nc.scalar.activation(a0[:], pg0[:], AF.Tanh, bias=bd_v[0])
                    b0 = actp.tile([128, TT], BF16, tag="b", name=f"b0_{s}")
                    nc.scalar.activation(b0[:], pg0[:], AF.Sigmoid,
                                         bias=bd_v[0])
                if h2:
                    s0_t = gtp.tile([128, TT], BF16, tag="s0", name=f"s0_{s}")
                    nc.vector.tensor_scalar(s0_t[:], st["psA"][:], bskip_v,
                                            0.0, ALU.add, ALU.max)
                wv.step(1)                                    # W3
                if h2:
                    ps5 = pgp.tile([128, TT], FP32, tag="ps", name=f"ps5_{s}")
                    for qq in range(2):
                        nc.tensor.matmul(ps5[64 * qq:64 * qq + 64, :],
                                         wsk1_s[:, qq, :], s0_t[:],
                                         start=True, stop=True)
                if h1:
                    g0 = gtp.tile([128, TT], BF16, tag="g0", name=f"g0_{s}")
                    nc.vector.tensor_mul(g0[:], a0[:], b0[:])
                wv.step(1)                                    # W4
                if h1:
                    # skip0 / res0 (skip stays accumulated in psA's bank)
                    psA = pgp.tile([128, TT], FP32, tag="ps", name=f"psA_{s}")
                    nc.tensor.matmul(psA[:], wsr_s[:, 0, :], g0[:],
                                     start=True, stop=False)
                    psB = pgp.tile([128, TT], FP32, tag="ps", name=f"psB_{s}")
                    nc.tensor.matmul(psB[:], wsr_s[:, 1, :], g0[:],
                                     start=True, stop=True)
                    state[s] = {"psA": psA}
                if h2:
                    if s % 2 == 0:
                        nc.vector.tensor_scalar(s1_s[:, u0:u0 + TT], ps5[:],
                                                bsk1_v, 0.0, ALU.add, ALU.max)
                    else:
                        nc.scalar.activation(s1_s[:, u0:u0 + TT], ps5[:],
                                             AF.Relu, bias=bsk1_v)
                    wv.push_tile(s - 1)
                wv.step(1)                                    # W5
                if h1:
                    nc.vector.scalar_tensor_tensor(
                        z1_s[:, 2 + t0:2 + t0 + TT], psB[:], bo0_v,
                        z0_s[:, 1 + t0:1 + t0 + TT], ALU.add, ALU.add)
                wv.step(2)                                    # W6
                state.pop(s - 1, None)

            weaver = Weaver()
            for s in range(NT + 1):
                emit_slot(s, weaver)
            weaver.flush()

    nc.compile()
    return nc


def get_nc():
    global _cached_nc
    if _cached_nc is None:
        _cached_nc = build_nc()
    return _cached_nc


def kernel(**inputs):
    nc = get_nc()
    w = prepare_weights(
        inputs["w_causal"], inputs["b_causal"],
        inputs["wd0"], inputs["bd0"], inputs["ws0"], inputs["bs0"],
        inputs["wo0"], inputs["bo0"],
        inputs["wd1"], inputs["bd1"], inputs["ws1"], inputs["bs1"],
        inputs["wo1"], inputs["bo1"],
        inputs["w_sk1"], inputs["b_sk1"], inputs["w_sk2"], inputs["b_sk2"])
    x = np.asarray(inputs["x"])
    in_maps = [{"xT": prepare_x(x, c), **w} for c in range(N_CORES)]
    res = run_bass_kernel_spmd(nc, in_maps, list(range(N_CORES)))
    out = np.concatenate(
        [np.asarray(res.results[c]["y"]).reshape(BPC, T, C_OUT)
         for c in range(N_CORES)], axis=0).astype(np.float32)
    out += _f(inputs["b_sk2"])[None, None, :]
    return out


# revision 41
# speedup vs baseline: 1.1479x; 1.1479x over previous
"""Trainium2 Bass kernel for the DiCNN (WaveNet-like) module.

Sharding: pure data parallelism — 4 batch items per core on 8 cores.
On-chip layout: channels on partitions, time on the free dim; the four
batch items are stacked as 4x32-partition bands (block-diag weights).

Structure: a software-pipelined tile-major loop over eight 512-wide
time tiles. The output stage of tile t-1 is woven through tile t's
body at every dependency stall point, so the PE always has long-ready
matmuls queued where it would otherwise idle waiting on the
scalar/vector activation chain. A dependency-free matmul warm-up burst
overlaps the input DMA-transposes to release the PE HAM clock gate
early.

PE-array tiling (the big cycle saves):
- The causal conv runs as 4 COL-TILED concurrent matmuls (one per
  batch, tile_position=(0, 32b)): each batch's input is staged as a
  128-partition stack [x_t | x_{t-1}], so one 512-cycle span computes
  what used to take four.
- The final 32->448 conv runs as ROW-TILED concurrent pairs: band q's
  s1 lives at partition strip 32q of one [128, T] tile, so bands can
  share the array in 32-row strips (K=32 each, tile_position=(32q, 0)),
  two at a time into one 2-bank PSUM tile.

The final conv is "flipped": stationary = s1 chunk [32, 128], moving =
w_sk2^T replicated on each 32-row strip [32, 448]; PSUM holds [t, co]
— the output layout. The stationary columns take times at stride 4
(t = tc0 + 4p + j for sub-chunk j), which makes a band's full 512-step
tile [128p, 4j, 448c] exactly match the row-major HBM layout of
y[q, tc0:tc0+512, :] — so each band-tile leaves in ONE store DMA (the
~600ns sync-engine cost per dma_start is flat in transfer size, so
store count is what matters). b_sk2 is added on the host (the bias row
would push K to 33, breaking the 32-row strip packing).

y is stored bf16 (halves store-DMA traffic) and upcast to fp32 on the
host. All matmul operands are bf16 (fp32 PSUM accumulation). x is
pre-transposed to channel-major bf16 on the host so input loads are
plain chunked DMAs (the HWDGE transpose path is ~4x slower and was
crowding the sync ring).
"""

import numpy as np
import ml_dtypes

import concourse.bacc as bacc
import concourse.tile as tile
from concourse import mybir
from concourse.bass_utils import run_bass_kernel_spmd

BF16 = mybir.dt.bfloat16
FP32 = mybir.dt.float32

B, T, C_IN, HID, C_OUT, K = 32, 4096, 64, 32, 448, 2
N_CORES = 8
BPC = B // N_CORES          # batches per core = 4
TT = 512                    # time-tile size
NT = T // TT                # 8 tiles
XROWS = 4112                # 4097 rounded up to a multiple of 16 (xbar rows)
DELTA = 1                   # output-stage pipeline delay in tiles
N_WARMUP = 0                # dependency-free warm-up matmuls

AF = mybir.ActivationFunctionType
ALU = mybir.AluOpType

_cached_nc = None


def _f(x):
    return np.asarray(x, dtype=np.float32)


def _bf(x):
    return np.asarray(x, dtype=np.float32).astype(ml_dtypes.bfloat16)


def _tile4(v):
    return np.tile(_f(v).reshape(-1), 4).reshape(128, 1)


def prepare_weights(w_causal, b_causal, wd0, bd0, ws0, bs0, wo0, bo0,
                    wd1, bd1, ws1, bs1, wo1, bo1, w_sk1, b_sk1, w_sk2, b_sk2):
    """Host-side weight layout transforms (identical for every core)."""
    del wo1, bo1, b_sk2  # wo1/bo1 dead; b_sk2 added host-side

    def diag4(w32):
        s = np.zeros((128, 128), np.float32)
        for i in range(4):
            s[32 * i:32 * i + 32, 32 * i:32 * i + 32] = w32
        return s

    # per-batch causal stationary [128, 32]: rows 0:64 act on x_t
    # (tap k=1), rows 64:128 on x_{t-1} (tap k=0)
    wc = np.zeros((128, 32), np.float32)
    wc[0:64, :] = _f(w_causal)[:, :, 1].T
    wc[64:128, :] = _f(w_causal)[:, :, 0].T

    wd = np.zeros((128, 4, 128), np.float32)
    for blk, w in enumerate((wd0, wd1)):
        for k in range(2):
            wd[:, 2 * blk + k, :] = diag4(_f(w)[:, :, k].T)

    wsr = np.zeros((128, 2, 128), np.float32)
    wsr[:, 0, :] = diag4(_f(ws0)[:, :, 0].T)
    wsr[:, 1, :] = diag4(_f(wo0)[:, :, 0].T)
    ws1d = diag4(_f(ws1)[:, :, 0].T)

    # wsk1[:, qq, :]: M=64 outputs -> s1 bands 2qq (cols 0:32) and
    # 2qq+1 (cols 32:64)
    wsk1 = np.zeros((128, 2, 64), np.float32)
    w1T = _f(w_sk1)[:, :, 0].T
    for qq in range(2):
        wsk1[64 * qq:64 * qq + 32, qq, 0:32] = w1T
        wsk1[64 * qq + 32:64 * qq + 64, qq, 32:64] = w1T

    # w2 replicated on every 32-row strip (row-tiled moving operand)
    w2 = np.zeros((128, 448), np.float32)
    for off in range(0, 128, 32):
        w2[off:off + 32, :] = _f(w_sk2)[:, :, 0].T

    bvecs = np.zeros((128, 6), np.float32)
    bvecs[:, 0] = _tile4(b_causal)[:, 0]
    bvecs[:, 1] = _tile4(bd0)[:, 0]
    bvecs[:, 2] = _tile4(bd1)[:, 0]
    bvecs[:, 3] = _tile4(bo0)[:, 0]
    bvecs[:, 4] = _tile4(_f(bs0) + _f(bs1))[:, 0]
    bvecs[:, 5] = _tile4(b_sk1)[:, 0]

    return dict(
        wc=_bf(wc), wd=_bf(wd), wsr=_bf(wsr), ws1d=_bf(ws1d),
        wsk1=_bf(wsk1), w2=_bf(w2), bvecs=np.ascontiguousarray(bvecs),
    )


def prepare_x(x, core):
    """Per-core channel-major input staging array [BPC, 128, XROWS] bf16.

    Column 1+t of batch b holds [x[b, t, :] | x[b, t-1, :]] — the causal
    two-tap stack — so the causal conv is a single K=128 matmul per
    batch, and the on-chip layout is loaded with plain (non-transposing)
    DMAs. Column 0 (and the t-1 half of column 1) are the causal pad.
    """
    xT = np.zeros((128, BPC, XROWS), ml_dtypes.bfloat16)
    xb = _bf(x)
    for b in range(BPC):
        xT[0:64, b, 1:1 + T] = xb[BPC * core + b].T
        xT[64:128, b, 2:1 + T] = xb[BPC * core + b][:-1].T
    return xT


def build_nc():
    nc = bacc.Bacc("TRN2", target_bir_lowering=False, debug=False,
                   num_devices=N_CORES)

    xT_d = nc.dram_tensor("xT", [128, BPC, XROWS], BF16,
                          kind="ExternalInput")
    wc_d = nc.dram_tensor("wc", [128, 32], BF16, kind="ExternalInput")
    wd_d = nc.dram_tensor("wd", [128, 4, 128], BF16, kind="ExternalInput")
    wsr_d = nc.dram_tensor("wsr", [128, 2, 128], BF16, kind="ExternalInput")
    ws1_d = nc.dram_tensor("ws1d", [128, 128], BF16, kind="ExternalInput")
    wsk1_d = nc.dram_tensor("wsk1", [128, 2, 64], BF16, kind="ExternalInput")
    w2_d = nc.dram_tensor("w2", [128, 448], BF16, kind="ExternalInput")
    bv_d = nc.dram_tensor("bvecs", [128, 6], FP32, kind="ExternalInput")
    # y[q, jt, p, j, c] is element (b=q, t=512*jt + 4*p + j, c) of the
    # [BPC, T, C_OUT] output — same bytes, viewed 5-D so a whole band-tile
    # leaves in one DMA.
    y_d = nc.dram_tensor("y", [BPC, NT, 128, 4, C_OUT], BF16,
                         kind="ExternalOutput")

    with tile.TileContext(nc) as tc:
        with (
            tc.tile_pool(name="const", bufs=1) as const,
            tc.tile_pool(name="persist", bufs=1) as persist,
            tc.tile_pool(name="act", bufs=3) as actp,
            tc.tile_pool(name="gtile", bufs=2) as gtp,
            tc.tile_pool(name="outbuf", bufs=4) as outbuf,
            tc.tile_pool(name="pg", bufs=4, space="PSUM") as pgp,
            tc.tile_pool(name="pout", bufs=2, space="PSUM") as poutp,
        ):
            # ---- constants (wd first — the warm-up burst needs it) ----
            wd_s = const.tile([128, 4, 128], BF16)
            nc.sync.dma_start(wd_s[:], wd_d.ap())
            wc_s = const.tile([128, 32], BF16)
            nc.sync.dma_start(wc_s[:], wc_d.ap())
            x_s = persist.tile([128, BPC, XROWS], BF16, tag="x",
                               name="x_all")
            nc.sync.dma_start(x_s[:, :, 0:2 * TT], xT_d[:, :, 0:2 * TT])
            wsr_s = const.tile([128, 2, 128], BF16)
            nc.sync.dma_start(wsr_s[:], wsr_d.ap())
            ws1_s = const.tile([128, 128], BF16)
            nc.sync.dma_start(ws1_s[:], ws1_d.ap())
            wsk1_s = const.tile([128, 2, 64], BF16)
            nc.sync.dma_start(wsk1_s[:], wsk1_d.ap())
            w2_s = const.tile([128, 448], BF16)
            nc.sync.dma_start(w2_s[:], w2_d.ap())
            bv_s = const.tile([128, 6], FP32)
            nc.sync.dma_start(bv_s[:], bv_d.ap())

            bcausal = bv_s[:, 0:1]
            bd_v = (bv_s[:, 1:2], bv_s[:, 2:3])
            bo0_v = bv_s[:, 3:4]
            bskip_v = bv_s[:, 4:5]
            bsk1_v = bv_s[:, 5:6]

            # ---- persistent activations ----
            # x loads are plain copies (host pre-transposed), chunked 1024
            # columns at a time; the first chunks were issued right after
            # wc/wd above so slot 0 starts as early as possible.
            for c0 in list(range(2 * TT, T, 2 * TT)) + [T]:
                cw = 2 * TT if c0 < T else 16
                nc.sync.dma_start(x_s[:, :, c0:c0 + cw],
                                  xT_d[:, :, c0:c0 + cw])
            z0_s = persist.tile([128, 4100], BF16, tag="z0")
            nc.vector.memset(z0_s[:, 0:1], 0.0)
            z1_s = persist.tile([128, 4100], BF16, tag="z1")
            nc.vector.memset(z1_s[:, 0:2], 0.0)
            # s1: band q on partition strip 32q:32q+32
            s1_s = persist.tile([128, T], BF16, tag="s1")

            # ---- PE warm-up burst (overlaps the x-transpose DMAs) ----
            wu_t = persist.tile([128, TT], BF16, tag="wu")
            nc.vector.memset(wu_t[:], 0.0)
            # preload the tanh/sigmoid activation tables so the ~1.3us
            # ACT_TABLE_LOADs happen during the input-DMA wait, not on
            # the first tile's critical chain
            pre_t = actp.tile([128, 1], BF16, tag="a", name="tbl_warm")
            nc.scalar.activation(pre_t[:], wu_t[:, 0:1], AF.Tanh)
            nc.scalar.activation(pre_t[:], wu_t[:, 0:1], AF.Sigmoid)
            hb_cnt = [0]

            def heartbeat(n):
                """Dependency-free PE filler matmuls: keep the HAM activity
                window busy across short dependency stalls so the 2.4 GHz
                clock state is never lost."""
                for _ in range(n):
                    pwu = poutp.tile([128, 2, TT], FP32, tag="po",
                                     name=f"pwu_{hb_cnt[0]}")
                    hb_cnt[0] += 1
                    nc.tensor.matmul(pwu[:, 0, :], wd_s[:, 0, :], wu_t[:],
                                     start=True, stop=True)

            heartbeat(N_WARMUP)

            # ---- woven output stage ------------------------------------
            # Source tile jt: 8 groups g = 4*bp + j (band-pair bp, chunk
            # j). Group g: 2 row-tiled concurrent MMs for bands 2bp,
            # 2bp+1, sub-chunk j: out rows t = tc0 + 4p + j.
            def emit_out_mms(jt, bp, j):
                tc0 = TT * jt
                po = poutp.tile([128, 2, TT], FP32, tag="po",
                                name=f"po_{jt}_{bp}_{j}")
                for i in range(2):
                    off = 32 * (2 * bp + i)
                    nc.tensor.matmul(
                        po[:, i, 0:C_OUT],
                        s1_s[off:off + 32, tc0 + j:tc0 + j + 509:4],
                        w2_s[off:off + 32, :], start=True, stop=True,
                        tile_position=(off, 0))
                return po

            def emit_out_drain(jt, bp, j, po, o_t, eng):
                if eng == 0:
                    nc.scalar.copy(o_t[:, j, :, :], po[:, :, 0:C_OUT])
                elif eng == 1:
                    nc.vector.tensor_copy(o_t[:, j, :, :], po[:, :, 0:C_OUT])
                else:
                    half = C_OUT // 2
                    nc.scalar.copy(o_t[:, j, :, 0:half], po[:, :, 0:half])
                    nc.vector.tensor_copy(o_t[:, j, :, half:C_OUT],
                                          po[:, :, half:C_OUT])
                if j == 3:
                    for i in range(2):
                        nc.sync.dma_start(y_d[2 * bp + i, jt],
                                          o_t[:, :, i, :])

            class Weaver:
                """Persistent output-stage scheduler. Tiles push their 8
                groups when their s1 write has been emitted; step() at a
                PE stall point enqueues the next group's matmuls, first
                emitting the cast of the group TWO steps back (its MMs
                are ~2.5us old by then, so the cast never stalls at the
                head of its engine queue). Casts alternate engines."""

                def __init__(self):
                    self.pend = []
                    self.next_mm = 0
                    self.next_drain = 0
                    self.inflight = {}
                    self.obufs = {}

                def push_tile(self, jt):
                    for j in range(4):
                        for bp in range(2):
                            self.pend.append((jt, bp, j))

                def _drain_until(self, stop, eng=None):
                    while self.next_drain < stop:
                        g = self.next_drain
                        jt, bp, j = self.pend[g]
                        emit_out_drain(jt, bp, j, self.inflight.pop(g),
                                       self.obufs[(jt, bp)],
                                       g % 2 if eng is None else eng)
                        if j == 3:
                            del self.obufs[(jt, bp)]
                        self.next_drain += 1

                def step(self, n=1):
                    for _ in range(n):
                        if self.next_mm >= len(self.pend):
                            return
                        # lag-2 drain; also satisfies the 2-buf pout pool
                        self._drain_until(max(self.next_drain,
                                              self.next_mm - 1))
                        g = self.next_mm
                        jt, bp, j = self.pend[g]
                        if j == 0:
                            self.obufs[(jt, bp)] = outbuf.tile(
                                [128, 4, 2, C_OUT], BF16, tag="o",
                                name=f"o_{jt}_{bp}")
                        self.inflight[g] = emit_out_mms(jt, bp, j)
                        self.next_mm += 1

                def idle(self):
                    return self.next_mm >= len(self.pend)

                def flush(self):
                    while self.next_mm < len(self.pend):
                        self.step(1)
                        self._drain_until(self.next_mm - 1, eng=2)
                    self._drain_until(self.next_mm, eng=2)

            # Two-stage software pipeline: slot s runs block-0 of tile s
            # (H1) interleaved with block-1+head of tile s-1 (H2) and the
            # woven output stage of tile s-2. Two dependency chains in
            # flight halve the wall-clock cost of the serial
            # z0->g0->z1->g1->s0->s1 chain.
            state = {}

            def emit_slot(s, wv):
                t0 = TT * s            # H1 tile offset
                u0 = TT * (s - 1)      # H2 tile offset
                h1 = s < NT
                h2 = 1 <= s <= NT
                st = state.get(s - 1)

                if h1:
                    # causal conv: 4 col-tiled concurrent MMs -> pz
                    pz = pgp.tile([128, TT], FP32, tag="ps", name=f"pz_{s}")
                    for b in range(BPC):
                        nc.tensor.matmul(pz[32 * b:32 * b + 32, :], wc_s[:],
                                         x_s[:, b, 1 + t0:1 + t0 + TT],
                                         start=True, stop=True,
                                         tile_position=(0, 32 * b))
                wv.step(1)                                    # W0
                if h2:
                    pg1 = pgp.tile([128, TT], FP32, tag="ps", name=f"pg1_{s}")
                    nc.tensor.matmul(pg1[:], wd_s[:, 2, :],
                                     z1_s[:, u0:u0 + TT],
                                     start=True, stop=False)
                    nc.tensor.matmul(pg1[:], wd_s[:, 3, :],
                                     z1_s[:, 2 + u0:2 + u0 + TT],
                                     start=False, stop=True)
                if h1:
                    nc.vector.tensor_scalar_add(z0_s[:, 1 + t0:1 + t0 + TT],
                                                pz[:], bcausal)
                if wv.idle():
                    heartbeat(4)
                wv.step(1)                                    # W1
                if h2:
                    a1 = actp.tile([128, TT], BF16, tag="a", name=f"a1_{s}")
                    nc.scalar.activation(a1[:], pg1[:], AF.Tanh, bias=bd_v[1])
                    b1 = actp.tile([128, TT], BF16, tag="b", name=f"b1_{s}")
                    nc.scalar.activation(b1[:], pg1[:], AF.Sigmoid,
                                         bias=bd_v[1])
                if h1:
                    pg0 = pgp.tile([128, TT], FP32, tag="ps", name=f"pg0_{s}")
                    nc.tensor.matmul(pg0[:], wd_s[:, 0, :],
                                     z0_s[:, t0:t0 + TT],
                                     start=True, stop=False)
                    nc.tensor.matmul(pg0[:], wd_s[:, 1, :],
                                     z0_s[:, 1 + t0:1 + t0 + TT],
                                     start=False, stop=True)
                if h2:
                    g1 = gtp.tile([128, TT], BF16, tag="g1", name=f"g1_{s}")
                    nc.vector.tensor_mul(g1[:], a1[:], b1[:])
                wv.step(1)                                    # W2
                if h2:
                    # head: s0 = relu(skip0 + ws1@g1 + bias); ws1 matmul
                    # accumulates onto psA's bank
                    nc.tensor.matmul(st["psA"][:], ws1_s[:], g1[:],
                                     start=False, stop=True)
                if h1:
                    a0 = actp.tile([128, TT], BF16, tag="a", name=f"a0_{s}")
                    nc.scalar.activation(a0[:], pg0[:], AF.Tanh, bias=bd_v[0])
                    b0 = actp.tile([128, TT], BF16, tag="b", name=f"b0_{s}")
                    nc.scalar.activation(b0[:], pg0[:], AF.Sigmoid,
                                         bias=bd_v[0])
                if h2:
                    s0_t = gtp.tile([128, TT], BF16, tag="s0", name=f"s0_{s}")
                    nc.vector.tensor_scalar(s0_t[:], st["psA"][:], bskip_v,
                                            0.0, ALU.add, ALU.max)
                wv.step(1)                                    # W3
                if h2:
                    ps5 = pgp.tile([128, TT], FP32, tag="ps", name=f"ps5_{s}")
                    for qq in range(2):
                        nc.tensor.matmul(ps5[64 * qq:64 * qq + 64, :],
                                         wsk1_s[:, qq, :], s0_t[:],
                                         start=True, stop=True)
                if h1:
                    g0 = gtp.tile([128, TT], BF16, tag="g0", name=f"g0_{s}")
                    nc.vector.tensor_mul(g0[:], a0[:], b0[:])
                wv.step(1)                                    # W4
                if h1:
                    # skip0 / res0 (skip stays accumulated in psA's bank)
                    psA = pgp.tile([128, TT], FP32, tag="ps", name=f"psA_{s}")
                    nc.tensor.matmul(psA[:], wsr_s[:, 0, :], g0[:],
                                     start=True, stop=False)
                    psB = pgp.tile([128, TT], FP32, tag="ps", name=f"psB_{s}")
                    nc.tensor.matmul(psB[:], wsr_s[:, 1, :], g0[:],
                                     start=True, stop=True)
                    state[s] = {"psA": psA}
                if h2:
                    if s % 2 == 0:
                        nc.vector.tensor_scalar(s1_s[:, u0:u0 + TT], ps5[:],
                                                bsk1_v, 0.0, ALU.add, ALU.max)
                    else:
                        nc.scalar.activation(s1_s[:, u0:u0 + TT], ps5[:],
                                             AF.Relu, bias=bsk1_v)
                    wv.push_tile(s - 1)
                wv.step(1)                                    # W5
                if h1:
                    nc.vector.scalar_tensor_tensor(
                        z1_s[:, 2 + t0:2 + t0 + TT], psB[:], bo0_v,
                        z0_s[:, 1 + t0:1 + t0 + TT], ALU.add, ALU.add)
                wv.step(2)                                    # W6
                state.pop(s - 1, None)

            weaver = Weaver()
            for s in range(NT + 1):
                emit_slot(s, weaver)
            weaver.flush()

    nc.compile()
    return nc


def get_nc():
    global _cached_nc
    if _cached_nc is None:
        _cached_nc = build_nc()
    return _cached_nc


def kernel(**inputs):
    nc = get_nc()
    w = prepare_weights(
        inputs["w_causal"], inputs["b_causal"],
        inputs["wd0"], inputs["bd0"], inputs["ws0"], inputs["bs0"],
        inputs["wo0"], inputs["bo0"],
        inputs["wd1"], inputs["bd1"], inputs["ws1"], inputs["bs1"],
        inputs["wo1"], inputs["bo1"],
        inputs["w_sk1"], inputs["b_sk1"], inputs["w_sk2"], inputs["b_sk2"])
    x = np.asarray(inputs["x"])
    in_maps = [{"xT": prepare_x(x, c), **w} for c in range(N_CORES)]
    res = run_bass_kernel_spmd(nc, in_maps, list(range(N_CORES)))
    out = np.concatenate(
        [np.asarray(res.results[c]["y"]).reshape(BPC, T, C_OUT)
         for c in range(N_CORES)], axis=0).astype(np.float32)
    out += _f(inputs["b_sk2"])[None, None, :]
    return out
